# revision 11
# baseline (speedup 1.0000x reference)
"""Trainium2 Bass kernel for nn_DGODE (graph ODE over utterance nodes).

Self-contained: hardcodes all shapes. Strategy:
- Row-shard B=4096 nodes over 8 cores (512 rows each).
- The unnormalized adjacency S is symmetric and exp(-0.1|i-j|) decays so
  fast that entries with |i-j| > 256 are < 1.2e-9 of the row sum: each
  core builds only a banded window S[jwin, rows_c] (jwin = 1024 rows
  around its block) directly in transposed orientation, with the row
  normalization folded in. It stays SBUF-resident for all 16 RK4 evals.
- Per ODE eval: AllGather y (row-form, f32r) -> banded matmul
  hnT = y[jwin].T @ S -> MLP in transposed orientation (all matmuls
  N=512) -> RK4 combination on DVE -> PE-transpose back to row-form.
"""

import sys

if "/opt/trn_rl_repo" not in sys.path:
    sys.path.insert(0, "/opt/trn_rl_repo")

import numpy as np

import concourse.bacc as bacc
import concourse.bass as bass
import concourse.mybir as mybir
import concourse.tile as tile
from concourse.bass_utils import run_bass_kernel_spmd

F32 = mybir.dt.float32
F32R = mybir.dt.float32r
U32 = mybir.dt.uint32
AF = mybir.ActivationFunctionType
ALU = mybir.AluOpType

NCORES = 8
B = 4096
D_IN = 1856
D_PAD = 1920           # 15 * 128
ND = D_PAD // 128      # 15 chunks of the input-projection contraction
H = 128
R = B // NCORES        # 512 rows per core
P = 128
NW = 8                 # window chunks
WIN = NW * P           # 1024-row banded window
N_STEPS = 4
DT = 1.0 / N_STEPS
A1, A2, BETA = 0.8, 0.5, 0.1

ROW_BASE = [min(max(R * c - 256, 0), B - WIN) for c in range(NCORES)]

_CACHED_NC = None


def build_nc():
    nc = bacc.Bacc(
        "TRN2",
        target_bir_lowering=False,
        debug=False,
        enable_asserts=True,
        num_devices=NCORES,
    )

    # ---- per-core external inputs ----
    xT_d = nc.dram_tensor("xT", [D_PAD, R], F32R, kind="ExternalInput")
    wp_d = nc.dram_tensor("wp", [D_PAD, H], F32, kind="ExternalInput")
    bp_d = nc.dram_tensor("bp", [H, 1], F32, kind="ExternalInput")
    w1_d = nc.dram_tensor("w1", [2 * H, H], F32, kind="ExternalInput")
    b1_d = nc.dram_tensor("b1", [H, 1], F32, kind="ExternalInput")
    w2_d = nc.dram_tensor("w2", [H, H], F32, kind="ExternalInput")
    b2_d = nc.dram_tensor("b2", [H, 1], F32, kind="ExternalInput")
    ident_d = nc.dram_tensor("ident", [P, P], F32, kind="ExternalInput")
    # replicated along partitions, free dim = this core's 512 rows (i)
    iidx_d = nc.dram_tensor("iidx", [P, R], F32, kind="ExternalInput")
    spki_d = nc.dram_tensor("spki", [P, R], F32, kind="ExternalInput")
    ai_d = nc.dram_tensor("ai", [P, R], F32, kind="ExternalInput")
    bi_d = nc.dram_tensor("bi", [P, R], F32, kind="ExternalInput")
    ci_d = nc.dram_tensor("ci", [P, R], F32, kind="ExternalInput")
    svi_d = nc.dram_tensor("svi", [P, R], F32, kind="ExternalInput")
    # window-j quantities, partition-major: [:, k][p] = value at j = row_base + 128k + p
    njw_d = nc.dram_tensor("njw", [P, NW], F32, kind="ExternalInput")   # -j
    jw_d = nc.dram_tensor("jw", [P, NW], F32, kind="ExternalInput")     # +j
    spkj_d = nc.dram_tensor("spkj", [P, NW], F32, kind="ExternalInput")
    aj_d = nc.dram_tensor("aj", [P, NW], F32, kind="ExternalInput")
    bj_d = nc.dram_tensor("bj", [P, NW], F32, kind="ExternalInput")
    cj_d = nc.dram_tensor("cj", [P, NW], F32, kind="ExternalInput")
    svj_d = nc.dram_tensor("svj", [P, NW], F32, kind="ExternalInput")
    rbc_d = nc.dram_tensor("rbc", [1, 1], U32, kind="ExternalInput")    # row_base/128

    out_d = nc.dram_tensor("hT_out", [H, R], F32, kind="ExternalOutput")

    with tile.TileContext(nc) as tc:
        with (
            tc.tile_pool(name="consts", bufs=1) as cs,
            tc.tile_pool(name="work", bufs=2) as wk,
            tc.tile_pool(name="states", bufs=1) as st,
            tc.tile_pool(name="ps", bufs=4, space="PSUM") as ps,
            tc.tile_pool(name="pst", bufs=2, space="PSUM") as pst,
            tc.tile_pool(name="dram", bufs=1, space="DRAM") as dram,
        ):
            # ---------- load constants ----------
            def load_const(dram_t, shape, name, rdtype=None):
                t = cs.tile(shape, F32, tag=name)
                nc.sync.dma_start(t[:], dram_t[:])
                if rdtype is None:
                    return t
                tr = cs.tile(shape, rdtype, tag=name + "_r")
                nc.vector.tensor_copy(tr[:], t[:])
                return tr

            wp_f = cs.tile([P, ND, H], F32, tag="wp_f")
            nc.sync.dma_start(wp_f[:], wp_d[:].rearrange("(n p) m -> p n m", p=P))
            wp_r = cs.tile([P, ND, H], F32R, tag="wp_r")
            nc.vector.tensor_copy(wp_r[:], wp_f[:])

            w1_f = cs.tile([P, 2, H], F32, tag="w1_f")
            nc.sync.dma_start(w1_f[:], w1_d[:].rearrange("(n p) m -> p n m", p=P))
            w1_r = cs.tile([P, 2, H], F32R, tag="w1_r")
            nc.vector.tensor_copy(w1_r[:], w1_f[:])

            w2_r = load_const(w2_d, [H, H], "w2", F32R)
            ident_r = load_const(ident_d, [P, P], "ident", F32R)
            bp_c = load_const(bp_d, [H, 1], "bp")
            b1_c = load_const(b1_d, [H, 1], "b1")
            b2_c = load_const(b2_d, [H, 1], "b2")

            iidx = load_const(iidx_d, [P, R], "iidx")
            spki = load_const(spki_d, [P, R], "spki")
            ai = load_const(ai_d, [P, R], "ai")
            bi = load_const(bi_d, [P, R], "bi")
            ci = load_const(ci_d, [P, R], "ci")
            svi = load_const(svi_d, [P, R], "svi")
            njw = load_const(njw_d, [P, NW], "njw")
            jw = load_const(jw_d, [P, NW], "jw")
            spkj = load_const(spkj_d, [P, NW], "spkj")
            aj = load_const(aj_d, [P, NW], "aj")
            bj = load_const(bj_d, [P, NW], "bj")
            cj = load_const(cj_d, [P, NW], "cj")
            svj = load_const(svj_d, [P, NW], "svj")

            rbc_sb = cs.tile([1, 1], U32, tag="rbc")
            nc.sync.dma_start(rbc_sb[:], rbc_d[:])
            regs = nc.alloc_registers("rbreg")
            nc.regs_load(regs, rbc_sb[0:1, 0:1])
            rbv = nc.snap(regs, donate=True)

            # ---------- input projection: hT = (X @ Wp).T + bp ----------
            xT_r = st.tile([P, ND, R], F32R, tag="xT_r")
            nc.sync.dma_start(xT_r[:], xT_d[:].rearrange("(n p) m -> p n m", p=P))

            h0_ps = ps.tile([P, R], F32, tag="ps")
            for d in range(ND):
                nc.tensor.matmul(
                    h0_ps[:], wp_r[:, d, :], xT_r[:, d, :],
                    start=(d == 0), stop=(d == ND - 1),
                )
            hT = st.tile([P, R], F32, tag="hT")
            nc.scalar.activation(hT[:], h0_ps[:], AF.Identity, bias=bp_c[:], scale=1.0)
            hT_r = st.tile([P, R], F32R, tag="hT_r")
            nc.scalar.activation(hT_r[:], h0_ps[:], AF.Identity, bias=bp_c[:], scale=1.0)

            # ---------- banded adjacency build (transposed, unnormalized) ----------
            s_tiles = []
            for k in range(NW):
                adt = wk.tile([P, R], F32, tag="adt")
                nc.scalar.activation(adt[:], iidx[:], AF.Abs,
                                     bias=njw[:, k : k + 1], scale=1.0)
                Tt = wk.tile([P, R], F32, tag="Tt")
                nc.scalar.activation(Tt[:], adt[:], AF.Exp, scale=-BETA)

                Pm = wk.tile([P, R], F32, tag="Pm")
                nc.vector.tensor_scalar(Pm[:], spki[:], spkj[:, k : k + 1], None,
                                        ALU.is_equal)
                m1 = wk.tile([P, R], F32, tag="m1")
                nc.vector.tensor_scalar(m1[:], ai[:], aj[:, k : k + 1], None, ALU.min)
                m2 = wk.tile([P, R], F32, tag="m2")
                nc.vector.scalar_tensor_tensor(m2[:], bi[:], bj[:, k : k + 1], m1[:],
                                               ALU.min, ALU.add)
                m3 = wk.tile([P, R], F32, tag="m3")
                nc.vector.scalar_tensor_tensor(m3[:], ci[:], cj[:, k : k + 1], m2[:],
                                               ALU.min, ALU.add)
                u0 = wk.tile([P, R], F32, tag="u0")
                nc.vector.scalar_tensor_tensor(u0[:], m3[:], 2.0 * A2 / 3.0, svi[:],
                                               ALU.mult, ALU.add)
                uu = wk.tile([P, R], F32, tag="uu")
                nc.vector.tensor_scalar(uu[:], u0[:], svj[:, k : k + 1], None,
                                        ALU.subtract)
                t1 = wk.tile([P, R], F32, tag="t1")
                nc.vector.tensor_scalar(t1[:], uu[:], -1.0, A1, ALU.mult, ALU.add)
                t2 = wk.tile([P, R], F32, tag="t2")
                nc.gpsimd.tensor_tensor(t2[:], Pm[:], t1[:], ALU.mult)
                qq = wk.tile([P, R], F32, tag="qq")
                nc.vector.tensor_tensor(qq[:], uu[:], t2[:], ALU.add)
                s0 = wk.tile([P, R], F32, tag="s0")
                nc.vector.tensor_tensor(s0[:], Tt[:], qq[:], ALU.mult)
                dm = wk.tile([P, R], F32, tag="dm")
                nc.vector.tensor_scalar(dm[:], iidx[:], jw[:, k : k + 1], 1.0 - A1,
                                        ALU.is_equal, ALU.mult)
                sk = cs.tile([P, R], F32R, tag=f"sk{k}")
                nc.vector.tensor_tensor(sk[:], s0[:], dm[:], ALU.add)
                s_tiles.append(sk)

            # ---------- row sums d_i, reciprocal, fold into S ----------
            ones_f = cs.tile([P, 1], F32, tag="ones_f")
            nc.vector.memset(ones_f[:], 1.0)
            ones_r = cs.tile([P, 1], F32R, tag="ones")
            nc.vector.tensor_copy(ones_r[:], ones_f[:])
            d_ps = ps.tile([P, R], F32, tag="ps")
            for k in range(NW):
                nc.tensor.matmul(d_ps[0:1, :], ones_r[:], s_tiles[k][:],
                                 start=(k == 0), stop=(k == NW - 1))
            dsum = cs.tile([1, R], F32, tag="dsum")
            nc.vector.tensor_scalar(dsum[:], d_ps[0:1, :], 1e-8, None, ALU.add)
            rd = cs.tile([1, R], F32R, tag="rd")
            with nc.allow_low_precision(reason="f32r is full-width storage"):
                nc.vector.reciprocal(rd[:], dsum[:])
            onesrow_f = cs.tile([1, P], F32, tag="onesrow_f")
            nc.vector.memset(onesrow_f[:], 1.0)
            onesrow_r = cs.tile([1, P], F32R, tag="onesrow")
            nc.vector.tensor_copy(onesrow_r[:], onesrow_f[:])
            rdb_ps = ps.tile([P, R], F32, tag="ps")
            nc.tensor.matmul(rdb_ps[:], onesrow_r[:], rd[:])
            for k in range(NW):
                nc.vector.tensor_tensor(s_tiles[k][:], s_tiles[k][:], rdb_ps[:],
                                        ALU.mult)

            # ---------- AG buffers ----------
            ag_in = dram.tile([R, H], F32R, tag="ag_in")

            agin_v = ag_in[:].rearrange("(n p) m -> p n m", p=P)

            def emit_transpose_to_agin(src_r):
                """src_r: [128, 512] f32r transposed-form tile -> ag_in row-form."""
                tp = pst.tile([P, 4, P], F32R, tag="tp")
                for t in range(4):
                    nc.tensor.transpose(tp[:, t, :], src_r[:, t * P : (t + 1) * P],
                                        ident_r[:])
                yrow = wk.tile([P, 4, P], F32R, tag="yrow")
                nc.vector.tensor_copy(yrow[:], tp[:])
                nc.sync.dma_start(agin_v, yrow[:])

            emit_transpose_to_agin(hT_r)

            # ---------- RK4 loop: 16 ODE evaluations ----------
            acc = st.tile([P, R], F32, tag="acc")
            for it in range(16):
                sub = it % 4
                y_r = hT_r if sub == 0 else cur_y  # noqa: F821

                ag_out = dram.tile([B, H], F32R, tag=f"ago{it}", addr_space="Shared")
                nc.gpsimd.collective_compute(
                    "AllGather",
                    ALU.bypass,
                    replica_groups=[list(range(NCORES))],
                    ins=[ag_in[:].opt()],
                    outs=[ag_out[:].opt()],
                )
                agv = ag_out[:].rearrange("(n p) m -> p n m", p=P)
                ywin = wk.tile([P, NW, P], F32R, tag="ywin")
                nc.sync.dma_start(ywin[:], agv[:, bass.ds(rbv, NW), :])

                hn_ps = ps.tile([P, R], F32, tag="ps")
                for k in range(NW):
                    nc.tensor.matmul(hn_ps[:], ywin[:, k, :], s_tiles[k][:],
                                     start=(k == 0), stop=(k == NW - 1))
                hn_r = wk.tile([P, R], F32R, tag="hn_r")
                nc.scalar.activation(hn_r[:], hn_ps[:], AF.Copy, bias=0.0, scale=1.0)

                z1_ps = ps.tile([P, R], F32, tag="ps")
                nc.tensor.matmul(z1_ps[:], w1_r[:, 0, :], y_r[:], start=True, stop=False)
                nc.tensor.matmul(z1_ps[:], w1_r[:, 1, :], hn_r[:], start=False, stop=True)
                th_r = wk.tile([P, R], F32R, tag="th_r")
                nc.scalar.activation(th_r[:], z1_ps[:], AF.Tanh, bias=b1_c[:], scale=1.0)

                z2_ps = ps.tile([P, R], F32, tag="ps")
                nc.tensor.matmul(z2_ps[:], w2_r[:], th_r[:])

                # k_i = z2 + b2 ; RK4 combination (transposed form, f32 state)
                kt = wk.tile([P, R], F32, tag="kt")
                nc.vector.tensor_scalar(kt[:], z2_ps[:], b2_c[:], None, ALU.add)
                if sub == 0:
                    nc.vector.tensor_copy(acc[:], kt[:])
                elif sub in (1, 2):
                    nc.vector.scalar_tensor_tensor(acc[:], kt[:], 2.0, acc[:],
                                                   ALU.mult, ALU.add)
                else:
                    nc.vector.tensor_tensor(acc[:], acc[:], kt[:], ALU.add)

                if sub < 3:
                    coef = 0.5 * DT if sub < 2 else DT
                    tmp = wk.tile([P, R], F32, tag="tmp")
                    nc.vector.tensor_scalar(tmp[:], kt[:], coef, None, ALU.mult)
                    cur_y = wk.tile([P, R], F32R, tag="cur_y")
                    nc.vector.tensor_tensor(cur_y[:], tmp[:], hT[:], ALU.add)
                    if it < 15:
                        emit_transpose_to_agin(cur_y)
                else:
                    tmp = wk.tile([P, R], F32, tag="tmp")
                    nc.vector.tensor_scalar(tmp[:], acc[:], DT / 6.0, None, ALU.mult)
                    hT_new = st.tile([P, R], F32, tag=f"hT_new{it}")
                    nc.vector.tensor_tensor(hT_new[:], tmp[:], hT[:], ALU.add)
                    hT = hT_new
                    if it < 15:
                        hT_r = st.tile([P, R], F32R, tag=f"hT_r{it}")
                        nc.vector.tensor_copy(hT_r[:], hT[:])
                        emit_transpose_to_agin(hT_r)

            nc.sync.dma_start(out_d[:], hT[:])

    nc.compile()
    return nc


def get_nc():
    global _CACHED_NC
    if _CACHED_NC is None:
        _CACHED_NC = build_nc()
    return _CACHED_NC


def prep_inputs(features, speaker_ids, modality_masks, Wp, bp, W1, b1, W2, b2):
    features = np.asarray(features, dtype=np.float32)
    spk = np.asarray(speaker_ids).astype(np.float32)
    mm = np.asarray(modality_masks, dtype=np.float32)
    Wp = np.asarray(Wp, dtype=np.float32)
    bp = np.asarray(bp, dtype=np.float32)
    W1 = np.asarray(W1, dtype=np.float32)
    b1 = np.asarray(b1, dtype=np.float32)
    W2 = np.asarray(W2, dtype=np.float32)
    b2 = np.asarray(b2, dtype=np.float32)

    wp_pad = np.zeros((D_PAD, H), dtype=np.float32)
    wp_pad[:D_IN] = Wp
    s_all = mm.sum(axis=1)           # s_i = sum_c m[i,c]
    sv_all = (A2 - (A2 / 3.0) * s_all).astype(np.float32)
    svj_all = ((A2 / 3.0) * s_all).astype(np.float32)
    ident = np.eye(P, dtype=np.float32)

    def rep(v):  # replicate a [R]-vector across partitions
        return np.ascontiguousarray(np.broadcast_to(v, (P, v.shape[0])), dtype=np.float32)

    def pm(v):   # [WIN] -> [P, NW] partition-major
        return np.ascontiguousarray(v.reshape(NW, P).T, dtype=np.float32)

    in_maps = []
    for c in range(NCORES):
        rows = slice(c * R, (c + 1) * R)
        rb = ROW_BASE[c]
        jwin = np.arange(rb, rb + WIN)
        xT = np.zeros((D_PAD, R), dtype=np.float32)
        xT[:D_IN] = features[rows].T
        ivals = np.arange(c * R, (c + 1) * R).astype(np.float32)
        in_maps.append({
            "xT": xT,
            "wp": wp_pad,
            "bp": bp.reshape(H, 1).copy(),
            "w1": W1.copy(),
            "b1": b1.reshape(H, 1).copy(),
            "w2": W2.copy(),
            "b2": b2.reshape(H, 1).copy(),
            "ident": ident,
            "iidx": rep(ivals),
            "spki": rep(spk[rows]),
            "ai": rep(mm[rows, 0]),
            "bi": rep(mm[rows, 1]),
            "ci": rep(mm[rows, 2]),
            "svi": rep(sv_all[rows]),
            "njw": pm(-jwin.astype(np.float32)),
            "jw": pm(jwin.astype(np.float32)),
            "spkj": pm(spk[jwin]),
            "aj": pm(mm[jwin, 0]),
            "bj": pm(mm[jwin, 1]),
            "cj": pm(mm[jwin, 2]),
            "svj": pm(svj_all[jwin]),
            "rbc": np.array([[rb // P]], dtype=np.uint32),
        })
    return in_maps


def kernel(features, speaker_ids, modality_masks, Wp, bp, W1, b1, W2, b2,
           _runner=None):
    in_maps = prep_inputs(features, speaker_ids, modality_masks,
                          Wp, bp, W1, b1, W2, b2)
    nc = get_nc()
    if _runner is not None:
        results = _runner(nc, in_maps)
    else:
        results = run_bass_kernel_spmd(nc, in_maps, list(range(NCORES))).results
    out = np.concatenate([results[c]["hT_out"].T for c in range(NCORES)], axis=0)
    return np.ascontiguousarray(out, dtype=np.float32)
